# revision 24
# baseline (speedup 1.0000x reference)
"""Grouped GEMM (MoE routing) Trainium2 kernel.

Strategy: tensor-parallel shard of the output N dim across 8 NeuronCores.
Every core sees all T=8192 tokens and a 512-wide slice of every expert's
weights, so per-core work is identical regardless of segment sizes and a
single SPMD program (with the segment boundaries baked in as compile-time
constants) runs on all 8 cores.

Per core:  out_t[n, t] = sum_k w_t[e(t), k, n] * a_t[k, t]

All operands are bf16 (well within the 2e-2 rel-err budget): this halves
HBM traffic vs fp32 (~109 MB/core) and keeps the matmul at 1 cycle/row,
making the kernel compute-bound at ~437us/core (1.05M PE rows @ 2.4GHz).

DMA plan: every DRAM region is pre-tiled on the host into the exact SBUF
consumption order, so each dma_start is one fully-contiguous 0.25-2 MB
transfer (split by HW across all 16 SDMA engines). Three separate DMA
paths so nothing queues behind anything else: a-tiles on the sync HWDGE
ring, weight chunks on the scalar (ACT) HWDGE ring, output stores on
gpsimd (SWDGE). Weights load as 4 x 1MB chunks in an 8-buffer ring, so
prefetch is paced by compute progress ~2 segments ahead and the per-
switch HBM spike is 1MB, not 4MB.

Blocks of up to 1024 tokens are processed as two <=512 halves sharing
each stationary weight tile (halving LoadStationary count); the 4 n-chunks
x 2 halves occupy all 8 PSUM banks. A small lead block lets the first
matmul start ~8us in; a small tail block keeps the final copy+store off
the critical path.
"""

import numpy as np
from ml_dtypes import bfloat16

import concourse.bacc as bacc
import concourse.bass as bass
import concourse.mybir as mybir
import concourse.tile as tile
from concourse.bass_utils import run_bass_kernel_spmd

NC = 8          # NeuronCores
P = 128         # partitions
TB = 1024       # max token block (two <=512 halves -> 8 live PSUM banks)
KOC = 8         # k-chunks per a-tile / w-chunk DMA batch
LEAD = 256      # lead/tail block size

LAST_RESULT = {}


def _segments(seg_indptr, weight_indices, batch_size, T):
    """Token segments per reference semantics: token t uses expert slot
    clip(searchsorted(indptr, t, 'right')-1, 0, bs-1)."""
    seg = np.asarray(seg_indptr).astype(np.int64)
    widx = np.asarray(weight_indices).astype(np.int64)
    bs = int(batch_size)
    segs = []
    for e in range(bs):
        s = 0 if e == 0 else int(min(max(seg[e], 0), T))
        t = T if e == bs - 1 else int(min(max(seg[e + 1], 0), T))
        if t > s:
            segs.append((s, t, int(widx[e])))
    return segs


def _token_blocks(segs):
    """Split each segment into equal pieces of <=512 tokens (the PSUM
    moving limit), then pair consecutive pieces into blocks whose two
    halves share each stationary weight load. A small lead (first) and
    tail (last) piece stays unpaired for pipeline ramp/drain."""
    blocks = []  # (tstart, tlen, run_idx, halves[(off,len)])
    run = -1
    nseg = len(segs)
    for si, (s, t, _w) in enumerate(segs):
        run += 1
        ln = t - s
        p = s
        solo_head = solo_tail = None
        if si == 0 and ln >= 2 * LEAD:
            solo_head = (p, LEAD)
            p += LEAD
            ln -= LEAD
        if si == nseg - 1 and ln >= 2 * LEAD:
            solo_tail = (t - LEAD, LEAD)
            ln -= LEAD
        pieces = []
        npieces = max(1, -(-ln // 512))
        base, rem = divmod(ln, npieces)
        for i in range(npieces):
            L = base + (1 if i < rem else 0)
            if L > 0:
                pieces.append((p, L))
                p += L
        if solo_head:
            blocks.append((solo_head[0], solo_head[1], run,
                           [(0, solo_head[1])]))
        i = 0
        while i < len(pieces):
            if i + 1 < len(pieces):
                (s1, l1), (_s2, l2) = pieces[i], pieces[i + 1]
                blocks.append((s1, l1 + l2, run, [(0, l1), (l1, l2)]))
                i += 2
            else:
                s1, l1 = pieces[i]
                blocks.append((s1, l1, run, [(0, l1)]))
                i += 1
        if solo_tail:
            blocks.append((solo_tail[0], solo_tail[1], run,
                           [(0, solo_tail[1])]))
    return blocks


def _build_program(T, K, NS, EA, blocks):
    f32 = mybir.dt.float32
    bf16 = mybir.dt.bfloat16
    KO = K // P
    NB = NS // P
    koc_n = min(KOC, KO)
    NCB = KO // koc_n          # w chunks / a-tile batches per block
    WCH = koc_n * NS           # w chunk elems per partition

    nc = bacc.Bacc(None, target_bir_lowering=False)
    at = nc.declare_dram_parameter("at", [P, KO * T], bf16, isOutput=False)
    wt = nc.declare_dram_parameter("wt", [EA, P, KO * NS], bf16, isOutput=False)
    ot = nc.declare_dram_parameter("ot", [P, NB * T], bf16, isOutput=True)

    with tile.TileContext(nc) as tc:
        with (
            tc.tile_pool(name="wpool", bufs=2 * NCB) as wpool,
            tc.tile_pool(name="apool", bufs=6) as apool,
            tc.tile_pool(name="opool", bufs=2) as opool,
            tc.tile_pool(name="psum", bufs=8, space=bass.MemorySpace.PSUM) as psum_pool,
        ):
            cur_run = -1
            wchunks = None
            for (ts, L, run, halves) in blocks:
                if run != cur_run:
                    wchunks = []
                    for cb in range(NCB):
                        wc = wpool.tile([P, WCH], bf16, tag="w",
                                        name=f"w{cb}")
                        # contiguous 1MB on the scalar (ACT) HWDGE ring so
                        # weight prefetch never queues behind a-tiles
                        nc.scalar.dma_start(
                            out=wc[:, :],
                            in_=wt[run][:, cb * WCH:(cb + 1) * WCH],
                        )
                        wchunks.append(wc)
                    cur_run = run
                ptiles = [
                    [psum_pool.tile([P, hl], f32, tag="ps", name=f"ps{nb}h{h}",
                                    padded_shape=[P, 512])
                     for h, (hs, hl) in enumerate(halves)]
                    for nb in range(NB)
                ]
                # small blocks: one a DMA for the whole k range (avoids 4
                # serialized sub-MB transfers each paying ~2us fixed cost).
                # The lead block stays fine-grained so the very first matmul
                # only waits for a 0.5MB transfer.
                a_koc = KO if (L <= 256 and ts > 0) else koc_n
                for cb in range(KO // a_koc):
                    a_tile = apool.tile([P, a_koc * L], bf16, tag="a",
                                        name="a_tile",
                                        padded_shape=[P, koc_n * TB])
                    base = KO * ts + cb * a_koc * L
                    nc.sync.dma_start(
                        out=a_tile[:, :],
                        in_=at[:, base:base + a_koc * L],
                    )
                    for koi in range(a_koc):
                        ko = cb * a_koc + koi
                        for nb in range(NB):
                            wko = ko % koc_n
                            stat = wchunks[ko // koc_n][:, wko * NS + nb * P:
                                                        wko * NS + (nb + 1) * P]
                            for h, (hs, hl) in enumerate(halves):
                                nc.tensor.matmul(
                                    ptiles[nb][h][:, :],
                                    stat,
                                    a_tile[:, koi * L + hs:koi * L + hs + hl],
                                    start=(ko == 0),
                                    stop=(ko == KO - 1),
                                )
                o_tile = opool.tile([P, NB * L], bf16, tag="o", name="o_tile",
                                    padded_shape=[P, NB * TB])
                for nb in range(NB):
                    for h, (hs, hl) in enumerate(halves):
                        nc.vector.tensor_copy(
                            o_tile[:, nb * L + hs:nb * L + hs + hl],
                            ptiles[nb][h][:, :hl],
                        )
                # SWDGE so output stores don't share a FIFO with loads
                nc.gpsimd.dma_start(
                    out=ot[:, NB * ts:NB * (ts + L)],
                    in_=o_tile[:, :],
                )
    nc.compile()
    return nc


def kernel(a, b, c, seg_indptr, weight_indices, batch_size, **_):
    T, K = a.shape
    E, N, K2 = b.shape
    assert K == K2
    NS = N // NC
    KO = K // P
    NB = NS // P
    koc_n = min(KOC, KO)

    segs = _segments(seg_indptr, weight_indices, batch_size, T)
    blocks = _token_blocks(segs)
    run_experts = [w for (_, _, w) in segs]  # expert id per weight-load run
    EA = len(run_experts)

    # a: [T, K] fp32 -> bf16, pre-tiled to [P, KO*T] in consumption order
    abf = a.astype(bfloat16)
    at_kpt = np.ascontiguousarray(abf.T).reshape(KO, P, T)  # [ko, p, t]
    at_flat = np.empty((P, KO * T), dtype=bfloat16)
    for (ts, L, *_rest) in blocks:
        for cb in range(KO // koc_n):
            slab = at_kpt[cb * koc_n:(cb + 1) * koc_n, :, ts:ts + L]
            at_flat[:, KO * ts + cb * koc_n * L:KO * ts + (cb + 1) * koc_n * L] = \
                slab.transpose(1, 0, 2).reshape(P, koc_n * L)

    # b: [E, N, K] fp32 -> bf16 [E, NC, P(k%128), KO, NS(n)]
    bbf = b.astype(bfloat16)
    wt_all = np.ascontiguousarray(
        bbf.reshape(E, NC, NS, KO, P).transpose(0, 1, 4, 3, 2)
    ).reshape(E, NC, P, KO * NS)

    in_maps = []
    for j in range(NC):
        wtj = np.ascontiguousarray(wt_all[run_experts, j])  # [EA, P, KO*NS]
        in_maps.append({"at": at_flat, "wt": wtj})

    nc = _build_program(T, K, NS, EA, blocks)

    import os
    trace = bool(int(os.environ.get("BASS_KERNEL_TRACE", "0")))
    res = run_bass_kernel_spmd(nc, in_maps, list(range(NC)), trace=trace)
    LAST_RESULT["exec_time_ns"] = res.exec_time_ns
    LAST_RESULT["results"] = res

    out = np.empty((T, N), dtype=np.float32)
    for j in range(NC):
        otj = res.results[j]["ot"]  # [P, NB*T] bf16
        for (ts, L, *_rest) in blocks:
            arr = otj[:, NB * ts:NB * (ts + L)].reshape(P, NB, L)
            out[ts:ts + L, j * NS:(j + 1) * NS] = \
                arr.transpose(2, 1, 0).reshape(L, NS).astype(np.float32)
    return out


# revision 29
# speedup vs baseline: 1.0100x; 1.0100x over previous
"""Grouped GEMM (MoE routing) Trainium2 kernel.

Strategy: tensor-parallel shard of the output N dim across 8 NeuronCores.
Every core sees all T=8192 tokens and a 512-wide slice of every expert's
weights, so per-core work is identical regardless of segment sizes and a
single SPMD program (with the segment boundaries baked in as compile-time
constants) runs on all 8 cores.

Per core:  out_t[n, t] = sum_k w_t[e(t), k, n] * a_t[k, t]

All operands are bf16 (well within the 2e-2 rel-err budget): this halves
HBM traffic vs fp32 (~109 MB/core) and keeps the matmul at 1 cycle/row,
making the kernel compute-bound at ~437us/core (1.05M PE rows @ 2.4GHz).

DMA plan: every DRAM region is pre-tiled on the host into the exact SBUF
consumption order, so each dma_start is one fully-contiguous 0.25-2 MB
transfer (split by HW across all 16 SDMA engines). Three separate DMA
paths so nothing queues behind anything else: a-tiles on the sync HWDGE
ring, weight chunks on the scalar (ACT) HWDGE ring, output stores on
gpsimd (SWDGE). Weights load as 4 x 1MB chunks in an 8-buffer ring, so
prefetch is paced by compute progress ~2 segments ahead and the per-
switch HBM spike is 1MB, not 4MB.

Blocks of up to 1024 tokens are processed as two <=512 halves sharing
each stationary weight tile (halving LoadStationary count); the 4 n-chunks
x 2 halves occupy all 8 PSUM banks. A small lead block lets the first
matmul start ~8us in; a small tail block keeps the final copy+store off
the critical path.
"""

import numpy as np
from ml_dtypes import bfloat16

import concourse.bacc as bacc
import concourse.bass as bass
import concourse.mybir as mybir
import concourse.tile as tile
from concourse.bass_utils import run_bass_kernel_spmd

NC = 8          # NeuronCores
P = 128         # partitions
TB = 1024       # max token block (two <=512 halves -> 8 live PSUM banks)
KOC = 8         # k-chunks per a-tile / w-chunk DMA batch
LEAD = 256      # lead/tail block size

LAST_RESULT = {}


def _segments(seg_indptr, weight_indices, batch_size, T):
    """Token segments per reference semantics: token t uses expert slot
    clip(searchsorted(indptr, t, 'right')-1, 0, bs-1)."""
    seg = np.asarray(seg_indptr).astype(np.int64)
    widx = np.asarray(weight_indices).astype(np.int64)
    bs = int(batch_size)
    segs = []
    for e in range(bs):
        s = 0 if e == 0 else int(min(max(seg[e], 0), T))
        t = T if e == bs - 1 else int(min(max(seg[e + 1], 0), T))
        if t > s:
            segs.append((s, t, int(widx[e])))
    return segs


def _token_blocks(segs):
    """Split each segment into equal pieces of <=512 tokens (the PSUM
    moving limit), then pair consecutive pieces into blocks whose two
    halves share each stationary weight load. A small lead (first) and
    tail (last) piece stays unpaired for pipeline ramp/drain."""
    blocks = []  # (tstart, tlen, run_idx, halves[(off,len)])
    run = -1
    nseg = len(segs)
    for si, (s, t, _w) in enumerate(segs):
        run += 1
        ln = t - s
        p = s
        solo_head = solo_tail = None
        if si == 0 and ln >= 2 * LEAD:
            solo_head = (p, LEAD)
            p += LEAD
            ln -= LEAD
        if si == nseg - 1 and ln >= 2 * LEAD:
            solo_tail = (t - LEAD, LEAD)
            ln -= LEAD
        pieces = []
        npieces = max(1, -(-ln // 512))
        base, rem = divmod(ln, npieces)
        for i in range(npieces):
            L = base + (1 if i < rem else 0)
            if L > 0:
                pieces.append((p, L))
                p += L
        if solo_head:
            blocks.append((solo_head[0], solo_head[1], run,
                           [(0, solo_head[1])]))
        i = 0
        while i < len(pieces):
            if i + 1 < len(pieces):
                (s1, l1), (_s2, l2) = pieces[i], pieces[i + 1]
                blocks.append((s1, l1 + l2, run, [(0, l1), (l1, l2)]))
                i += 2
            else:
                s1, l1 = pieces[i]
                blocks.append((s1, l1, run, [(0, l1)]))
                i += 1
        if solo_tail:
            blocks.append((solo_tail[0], solo_tail[1], run,
                           [(0, solo_tail[1])]))
    return blocks


def _build_program(T, K, NS, EA, blocks):
    f32 = mybir.dt.float32
    bf16 = mybir.dt.bfloat16
    KO = K // P
    NB = NS // P
    koc_n = min(KOC, KO)
    NCB = KO // koc_n          # w chunks / a-tile batches per block
    WCH = koc_n * NS           # w chunk elems per partition

    nc = bacc.Bacc(None, target_bir_lowering=False)
    at = nc.declare_dram_parameter("at", [P, KO * T], bf16, isOutput=False)
    wt = nc.declare_dram_parameter("wt", [EA, P, KO * NS], bf16, isOutput=False)
    ot = nc.declare_dram_parameter("ot", [P, NB * T], bf16, isOutput=True)

    with tile.TileContext(nc) as tc:
        with (
            tc.tile_pool(name="wpool", bufs=2 * NCB) as wpool,
            tc.tile_pool(name="apool", bufs=6) as apool,
            tc.tile_pool(name="opool", bufs=2) as opool,
            tc.tile_pool(name="psum", bufs=8, space=bass.MemorySpace.PSUM) as psum_pool,
        ):
            cur_run = -1
            wchunks = None
            for (ts, L, run, halves) in blocks:
                if run != cur_run:
                    # contiguous 1MB chunks on the scalar (ACT) HWDGE ring so
                    # weight prefetch never queues behind a-tiles
                    run_cn = koc_n
                    wch = run_cn * NS
                    wchunks = []
                    for cb in range(KO // run_cn):
                        wc = wpool.tile([P, wch], bf16, tag="w",
                                        name=f"w{cb}", padded_shape=[P, WCH])
                        nc.scalar.dma_start(
                            out=wc[:, :],
                            in_=wt[run][:, cb * wch:(cb + 1) * wch],
                        )
                        wchunks.append(wc)
                    cur_run = run
                ptiles = [
                    [psum_pool.tile([P, hl], f32, tag="ps", name=f"ps{nb}h{h}",
                                    padded_shape=[P, 512])
                     for h, (hs, hl) in enumerate(halves)]
                    for nb in range(NB)
                ]
                # small blocks: one a DMA for the whole k range (avoids 4
                # serialized sub-MB transfers each paying ~2us fixed cost).
                # The lead block stays fine-grained so the very first matmul
                # only waits for a 0.5MB transfer.
                a_koc = KO if (L <= 256 and ts > 0) else koc_n
                for cb in range(KO // a_koc):
                    a_tile = apool.tile([P, a_koc * L], bf16, tag="a",
                                        name="a_tile",
                                        padded_shape=[P, koc_n * TB])
                    base = KO * ts + cb * a_koc * L
                    nc.sync.dma_start(
                        out=a_tile[:, :],
                        in_=at[:, base:base + a_koc * L],
                    )
                    for koi in range(a_koc):
                        ko = cb * a_koc + koi
                        for nb in range(NB):
                            wko = ko % run_cn
                            stat = wchunks[ko // run_cn][:, wko * NS + nb * P:
                                                         wko * NS + (nb + 1) * P]
                            for h, (hs, hl) in enumerate(halves):
                                nc.tensor.matmul(
                                    ptiles[nb][h][:, :],
                                    stat,
                                    a_tile[:, koi * L + hs:koi * L + hs + hl],
                                    start=(ko == 0),
                                    stop=(ko == KO - 1),
                                )
                o_tile = opool.tile([P, NB * L], bf16, tag="o", name="o_tile",
                                    padded_shape=[P, NB * TB])
                for nb in range(NB):
                    for h, (hs, hl) in enumerate(halves):
                        nc.vector.tensor_copy(
                            o_tile[:, nb * L + hs:nb * L + hs + hl],
                            ptiles[nb][h][:, :hl],
                        )
                # SWDGE so output stores don't share a FIFO with loads
                nc.gpsimd.dma_start(
                    out=ot[:, NB * ts:NB * (ts + L)],
                    in_=o_tile[:, :],
                )
    nc.compile()
    return nc


def kernel(a, b, c, seg_indptr, weight_indices, batch_size, **_):
    T, K = a.shape
    E, N, K2 = b.shape
    assert K == K2
    NS = N // NC
    KO = K // P
    NB = NS // P
    koc_n = min(KOC, KO)

    segs = _segments(seg_indptr, weight_indices, batch_size, T)
    blocks = _token_blocks(segs)
    run_experts = [w for (_, _, w) in segs]  # expert id per weight-load run
    EA = len(run_experts)

    # a: [T, K] fp32 -> bf16, pre-tiled to [P, KO*T] in consumption order
    abf = a.astype(bfloat16)
    at_kpt = np.ascontiguousarray(abf.T).reshape(KO, P, T)  # [ko, p, t]
    at_flat = np.empty((P, KO * T), dtype=bfloat16)
    for (ts, L, *_rest) in blocks:
        for cb in range(KO // koc_n):
            slab = at_kpt[cb * koc_n:(cb + 1) * koc_n, :, ts:ts + L]
            at_flat[:, KO * ts + cb * koc_n * L:KO * ts + (cb + 1) * koc_n * L] = \
                slab.transpose(1, 0, 2).reshape(P, koc_n * L)

    # b: [E, N, K] fp32 -> bf16 [E, NC, P(k%128), KO, NS(n)]
    bbf = b.astype(bfloat16)
    wt_all = np.ascontiguousarray(
        bbf.reshape(E, NC, NS, KO, P).transpose(0, 1, 4, 3, 2)
    ).reshape(E, NC, P, KO * NS)

    in_maps = []
    for j in range(NC):
        wtj = np.ascontiguousarray(wt_all[run_experts, j])  # [EA, P, KO*NS]
        in_maps.append({"at": at_flat, "wt": wtj})

    nc = _build_program(T, K, NS, EA, blocks)

    import os
    trace = bool(int(os.environ.get("BASS_KERNEL_TRACE", "0")))
    res = run_bass_kernel_spmd(nc, in_maps, list(range(NC)), trace=trace)
    LAST_RESULT["exec_time_ns"] = res.exec_time_ns
    LAST_RESULT["results"] = res

    out = np.empty((T, N), dtype=np.float32)
    for j in range(NC):
        otj = res.results[j]["ot"]  # [P, NB*T] bf16
        for (ts, L, *_rest) in blocks:
            arr = otj[:, NB * ts:NB * (ts + L)].reshape(P, NB, L)
            out[ts:ts + L, j * NS:(j + 1) * NS] = \
                arr.transpose(2, 1, 0).reshape(L, NS).astype(np.float32)
    return out
